# revision 6
# baseline (speedup 1.0000x reference)
"""Trainium2 Bass kernel for nn_MessagePassingNet (3-step GNN message passing).

Strategy (8 NeuronCores, SPMD):
- Atoms are range-partitioned: core k owns atoms [k*6250, (k+1)*6250).
  Every edge is assigned to the core owning its dst atom, so the per-step
  segment_sum over dst is core-local.
- Per step, the big per-edge input matmul is algebraically split into
  per-atom projections:  concat(s[dst], s[src]) @ Win
                         = (s @ Win_top)[dst] + (s @ Win_bot)[src]
  Each core projects only its own atoms (one 128x128 matmul per 128-atom
  block), then a single AllGather shares the src-side projection table.
- Per-edge work runs on 512-edge supertiles in feature-major layout, fed by
  transposed dma_gather (rows land as columns, no PE transposes anywhere).
  The segment_sum uses one-hot matmuls accumulating s^T blocks in PSUM.
- The final molecule segment_sum is another one-hot matmul per block, a tiny
  AllReduce, and a replicated 3-layer MLP.
"""
import sys

sys.path.insert(0, '/opt/trn_rl_repo')

import numpy as np
import ml_dtypes

import concourse.bacc as bacc
import concourse.bass as bass
import concourse.mybir as mybir
import concourse.tile as tile
from concourse.bass_utils import run_bass_kernel_spmd

BF16 = ml_dtypes.bfloat16
F32 = mybir.dt.float32
BF = mybir.dt.bfloat16
I16 = mybir.dt.int16
I32 = mybir.dt.int32
AF = mybir.ActivationFunctionType
ALU = mybir.AluOpType

NCORES = 8
ST = 4  # chunks (128 edges) per supertile


class Cfg:
    def __init__(self, n_atoms, n_edges, n_mols, steps=3):
        self.N = n_atoms
        self.E = n_edges
        self.MOLS = n_mols
        self.STEPS = steps
        self.D = 128
        self.H = 64
        self.OUT = 32
        assert self.N % NCORES == 0
        self.NLOC = self.N // NCORES
        self.NB = (self.NLOC + 127) // 128
        self.NLOCP = self.NB * 128
        self.ROWS = self.NLOCP + 1       # +1 zero row
        self.BROWS = self.ROWS * NCORES
        self.BHALF = self.ROWS * (NCORES // 2)
        assert self.BHALF <= 32767 and self.ROWS <= 32767


FULL = Cfg(50000, 400000, 256)


# ---------------------------------------------------------------- host prep

def _wrap_idx(a, n_st):
    """[EPC] int16 -> [128, n_st*32] wrapped (i at [i%16, i//16]) replicated x8."""
    a2 = a.reshape(n_st, 32, 16).transpose(1, 0, 2)   # [32, n_st, 16]
    w = a2.transpose(2, 1, 0).reshape(16, n_st * 32)  # [16p, n_st*32]
    return np.ascontiguousarray(np.tile(w, (8, 1)))


def preprocess(cfg, states, Win, b_in, Wh, b_h, Wout, b_out,
               fc1_w, fc1_b, fc2_w, fc2_b, out_w, out_b,
               src, dst, mol_ids):
    c = cfg
    src = np.asarray(src, np.int64)
    dst = np.asarray(dst, np.int64)
    mol_ids = np.asarray(mol_ids, np.int64)
    states = np.asarray(states, np.float32)

    sidx = np.lexsort((src, dst))
    s_dst = dst[sidx]
    s_src = src[sidx]
    core = s_dst // c.NLOC

    # global chunks-per-block
    loc_all = s_dst - core * c.NLOC
    blk_all = loc_all >> 7
    cnt = np.zeros((NCORES, c.NB), np.int64)
    np.add.at(cnt, (core, blk_all), 1)
    cpb = max(1, int(-(-cnt.max() // 128)))
    n_chunks = -(-(c.NB * cpb) // ST) * ST
    n_st = n_chunks // ST
    epc = n_chunks * 128

    per_core = []
    for k in range(NCORES):
        m = core == k
        lock = loc_all[m]
        bk = blk_all[m]
        relk = lock & 127
        srck = s_src[m]
        n = len(lock)
        ck = np.bincount(bk, minlength=c.NB)
        offs = np.r_[0, np.cumsum(ck)[:-1]]
        within = np.arange(n) - offs[bk]
        pos = bk * (cpb * 128) + within

        dst16 = np.zeros(epc, np.int16)
        dst16[pos] = 1 + lock
        slo16 = np.zeros(epc, np.int16)
        shi16 = np.zeros(epc, np.int16)
        score = srck // c.NLOC
        sloc = srck - score * c.NLOC
        lo = score < (NCORES // 2)
        slo16[pos[lo]] = (c.ROWS * score[lo] + 1 + sloc[lo]).astype(np.int16)
        shi16[pos[~lo]] = (c.ROWS * (score[~lo] - NCORES // 2) + 1
                           + sloc[~lo]).astype(np.int16)
        drel = np.full(epc, 300.0, np.float32)
        drel[pos] = relk

        mol_loc = np.full(c.NLOCP, 300.0, np.float32)
        mol_loc[:c.NLOC] = mol_ids[k * c.NLOC:(k + 1) * c.NLOC]

        sT = np.zeros((c.D, c.NLOCP), np.float32)
        sT[:, :c.NLOC] = states[k * c.NLOC:(k + 1) * c.NLOC].T

        per_core.append({
            "sT0": sT,
            "dsti": _wrap_idx(dst16, n_st),
            "slo": _wrap_idx(slo16, n_st),
            "shi": _wrap_idx(shi16, n_st),
            "drel": np.ascontiguousarray(drel.reshape(n_chunks, 128).T),
            "mols": np.ascontiguousarray(mol_loc.reshape(c.NB, 128).T),
        })

    Win = np.asarray(Win, np.float32)
    wcat = np.concatenate([Win[:, :c.D, :], Win[:, c.D:, :]], axis=2)  # [S,128,128]
    wouth = np.concatenate(
        [np.asarray(Wout, np.float32),
         np.asarray(b_out, np.float32)[:, None, :]], axis=1)           # [S,65,128]
    shared = {
        # packed [rows, steps*cols] so SBUF tiles load with a plain DMA
        "wcat": np.ascontiguousarray(
            wcat.transpose(1, 0, 2).reshape(c.D, -1).astype(np.float32)),
        "whl": np.ascontiguousarray(
            np.asarray(Wh, np.float32).transpose(1, 0, 2).reshape(c.H, -1)
        ).astype(BF16),
        "wouth": np.ascontiguousarray(
            wouth.transpose(1, 0, 2).reshape(c.H + 1, -1)).astype(BF16),
        "binT": np.ascontiguousarray(np.asarray(b_in, np.float32).T),   # [64,S]
        "bhT": np.ascontiguousarray(np.asarray(b_h, np.float32).T),     # [64,S]
        "fc1w": np.ascontiguousarray(np.asarray(fc1_w, np.float32).astype(BF16)),
        "fc2w": np.ascontiguousarray(np.asarray(fc2_w, np.float32).astype(BF16)),
        "outw": np.ascontiguousarray(np.asarray(out_w, np.float32).astype(BF16)),
        "fc1b": np.asarray(fc1_b, np.float32).reshape(-1, 1),
        "fc2b": np.asarray(fc2_b, np.float32).reshape(-1, 1),
        "outb": np.asarray(out_b, np.float32).reshape(-1, 1),
    }
    in_maps = [{**shared, **pc} for pc in per_core]
    return in_maps, cpb, n_chunks


# ---------------------------------------------------------------- program

def build_program(cfg, cpb, n_chunks, collectives=True, reps=1):
    c = cfg
    n_st = n_chunks // ST
    S = c.STEPS
    nc = bacc.Bacc("TRN2", target_bir_lowering=False, debug=False,
                   num_devices=NCORES)

    din = {}
    def inp(name, shape, dt):
        din[name] = nc.dram_tensor(name, shape, dt, kind="ExternalInput")
        return din[name]

    sT0 = inp("sT0", [c.D, c.NLOCP], F32)
    wcat = inp("wcat", [128, S * 128], F32)
    whl = inp("whl", [64, S * 64], BF)
    wouth = inp("wouth", [65, S * 128], BF)
    binT = inp("binT", [64, S], F32)
    bhT = inp("bhT", [64, S], F32)
    fc1w = inp("fc1w", [128, 64], BF)
    fc2w = inp("fc2w", [64, 64], BF)
    outw = inp("outw", [64, 32], BF)
    fc1b = inp("fc1b", [64, 1], F32)
    fc2b = inp("fc2b", [64, 1], F32)
    outb = inp("outb", [32, 1], F32)
    dsti = inp("dsti", [128, n_st * 32], I16)
    slo = inp("slo", [128, n_st * 32], I16)
    shi = inp("shi", [128, n_st * 32], I16)
    drel = inp("drel", [128, n_chunks], F32)
    mols = inp("mols", [128, c.NB], F32)

    y = nc.dram_tensor("y", [32, c.MOLS], F32, kind="ExternalOutput")

    A_loc = nc.dram_tensor("A_loc", [c.ROWS, 128], BF)
    B_shard = nc.dram_tensor("B_shard", [c.ROWS, 128], BF)
    B_full = nc.dram_tensor("B_full", [c.BROWS, 128], BF,
                            addr_space="Shared" if collectives else "Local")
    molpart = nc.dram_tensor("molpart", [128, c.MOLS], F32)
    molfull = nc.dram_tensor("molfull", [128, c.MOLS], F32,
                             addr_space="Shared" if collectives else "Local")

    with tile.TileContext(nc) as tc:
        with (
            tc.tile_pool(name="const", bufs=1) as cpool,
            tc.tile_pool(name="pph", bufs=2) as ppool,
            tc.tile_pool(name="edge", bufs=3) as epool,
            tc.tile_pool(name="gath", bufs=4) as gpool,
            tc.tile_pool(name="ohp", bufs=6) as ohpool,
            tc.tile_pool(name="blk", bufs=2) as bpool,
            tc.tile_pool(name="psA", bufs=2, space="PSUM") as psA,
            tc.tile_pool(name="psS", bufs=2, space="PSUM") as psS,
            tc.tile_pool(name="psM", bufs=1, space="PSUM") as psM,
        ):
            # ---------------- static tiles
            sT = cpool.tile([c.D, c.NLOCP], F32, tag="sT")
            nc.sync.dma_start(out=sT[:], in_=sT0[:])

            wcat_sb = cpool.tile([128, S * 128], F32, tag="wcat")
            nc.sync.dma_start(out=wcat_sb[:], in_=wcat[:])
            whl_sb = cpool.tile([64, S * 64], BF, tag="whl")
            nc.sync.dma_start(out=whl_sb[:], in_=whl[:])
            wouth_sb = cpool.tile([65, S * 128], BF, tag="wouth")
            nc.sync.dma_start(out=wouth_sb[:], in_=wouth[:])
            bin_sb = cpool.tile([64, S], F32, tag="bin")
            nc.sync.dma_start(out=bin_sb[:], in_=binT[:])
            bh_sb = cpool.tile([64, S], F32, tag="bh")
            nc.sync.dma_start(out=bh_sb[:], in_=bhT[:])
            fc1w_sb = cpool.tile([128, 64], BF, tag="fc1w")
            nc.sync.dma_start(out=fc1w_sb[:], in_=fc1w[:])
            fc2w_sb = cpool.tile([64, 64], BF, tag="fc2w")
            nc.sync.dma_start(out=fc2w_sb[:], in_=fc2w[:])
            outw_sb = cpool.tile([64, 32], BF, tag="outw")
            nc.sync.dma_start(out=outw_sb[:], in_=outw[:])
            fc1b_sb = cpool.tile([64, 1], F32, tag="fc1b")
            nc.sync.dma_start(out=fc1b_sb[:], in_=fc1b[:])
            fc2b_sb = cpool.tile([64, 1], F32, tag="fc2b")
            nc.sync.dma_start(out=fc2b_sb[:], in_=fc2b[:])
            outb_sb = cpool.tile([32, 1], F32, tag="outb")
            nc.sync.dma_start(out=outb_sb[:], in_=outb[:])

            dsti_sb = cpool.tile([128, n_st * 32], I16, tag="dsti")
            nc.sync.dma_start(out=dsti_sb[:], in_=dsti[:])
            slo_sb = cpool.tile([128, n_st * 32], I16, tag="slo")
            nc.sync.dma_start(out=slo_sb[:], in_=slo[:])
            shi_sb = cpool.tile([128, n_st * 32], I16, tag="shi")
            nc.sync.dma_start(out=shi_sb[:], in_=shi[:])
            drel_sb = cpool.tile([128, n_chunks], F32, tag="drel")
            nc.sync.dma_start(out=drel_sb[:], in_=drel[:])
            mols_sb = cpool.tile([128, c.NB], F32, tag="mols")
            nc.sync.dma_start(out=mols_sb[:], in_=mols[:])

            iota_i = cpool.tile([128, 256], I32, tag="iotai")
            nc.gpsimd.iota(iota_i[:], pattern=[[1, 256]], base=0,
                           channel_multiplier=0)
            iota128 = cpool.tile([128, 128], F32, tag="iota128")
            nc.vector.tensor_copy(out=iota128[:], in_=iota_i[:, :128])
            iota256 = cpool.tile([128, 256], F32, tag="iota256")
            nc.vector.tensor_copy(out=iota256[:], in_=iota_i[:])
            identity = cpool.tile([128, 128], BF, tag="ident")
            nc.gpsimd.memset(identity[:], 0.0)
            nc.gpsimd.affine_select(
                out=identity[:], in_=identity[:], compare_op=ALU.not_equal,
                fill=1.0, base=0, pattern=[[-1, 128]], channel_multiplier=1)

            zero_row = cpool.tile([1, 128], BF, tag="zrow")
            nc.vector.memset(zero_row[:], 0.0)
            nc.sync.dma_start(out=A_loc[0:1, :], in_=zero_row[:])
            nc.sync.dma_start(out=B_shard[0:1, :], in_=zero_row[:])

            m2h_a = cpool.tile([65, 512], BF, tag="m2h0")
            m2h_b = cpool.tile([65, 512], BF, tag="m2h1")
            m2h = [m2h_a, m2h_b]
            for t_ in m2h:
                nc.vector.memset(t_[64:65, :], 1.0)

            for _rep in range(reps):
                psmol = psM.tile([128, c.MOLS], F32, tag="mol")
                for t in range(S):
                    # ---------------- P phase
                    for b in range(c.NB):
                        psP = psA.tile([128, 128], F32, tag="m3")
                        nc.tensor.matmul(
                            out=psP[:],
                            lhsT=sT[:, 128 * b:128 * b + 128],
                            rhs=wcat_sb[:, 128 * t:128 * t + 128],
                            start=True, stop=True)
                        a_sb = ppool.tile([128, 128], BF, tag="a")
                        nc.vector.tensor_copy(out=a_sb[:], in_=psP[:])
                        b_sb = ppool.tile([128, 128], BF, tag="b")
                        nc.vector.tensor_copy(out=b_sb[:, 0:64], in_=psP[:, 64:128])
                        nc.vector.tensor_copy(out=b_sb[:, 64:128], in_=psP[:, 0:64])
                        r0 = 1 + 128 * b
                        nc.sync.dma_start(out=A_loc[r0:r0 + 128, :], in_=a_sb[:])
                        nc.sync.dma_start(out=B_shard[r0:r0 + 128, :], in_=b_sb[:])

                    # ---------------- AllGather of src-side table
                    if collectives:
                        nc.gpsimd.collective_compute(
                            "AllGather", ALU.bypass,
                            replica_groups=[list(range(NCORES))],
                            ins=[B_shard[:]], outs=[B_full[:]])
                    else:
                        nc.sync.dma_start(out=B_full[0:c.ROWS, :], in_=B_shard[:])

                    # ---------------- edge phase
                    for st in range(n_st):
                        gd = gpool.tile([128, 512], BF, tag="gd")
                        gl = gpool.tile([128, 512], BF, tag="gl")
                        gh = gpool.tile([128, 512], BF, tag="gh")
                        i0 = 32 * st
                        nc.gpsimd.dma_gather(
                            out_ap=gd[:].rearrange("p (a b) -> p a b", a=1),
                            in_ap=A_loc[:], idxs_ap=dsti_sb[:, i0:i0 + 32],
                            num_idxs=512, num_idxs_reg=512, elem_size=128,
                            transpose=True)
                        nc.gpsimd.dma_gather(
                            out_ap=gl[:].rearrange("p (a b) -> p a b", a=1),
                            in_ap=B_full[0:c.BHALF, :],
                            idxs_ap=slo_sb[:, i0:i0 + 32],
                            num_idxs=512, num_idxs_reg=512, elem_size=128,
                            transpose=True)
                        nc.gpsimd.dma_gather(
                            out_ap=gh[:].rearrange("p (a b) -> p a b", a=1),
                            in_ap=B_full[c.BHALF:c.BROWS, :],
                            idxs_ap=shi_sb[:, i0:i0 + 32],
                            num_idxs=512, num_idxs_reg=512, elem_size=128,
                            transpose=True)

                        s01 = epool.tile([64, 512], BF, tag="s01")
                        nc.vector.tensor_tensor(
                            out=s01[:], in0=gl[0:64, :], in1=gh[0:64, :],
                            op=ALU.add)
                        pre = epool.tile([64, 512], BF, tag="pre")
                        nc.vector.tensor_tensor(
                            out=pre[:], in0=s01[:], in1=gd[0:64, :], op=ALU.add)
                        m1 = epool.tile([64, 512], BF, tag="m1")
                        nc.vector.tensor_scalar(
                            m1[:], pre[:], bin_sb[:, t:t + 1], 0.0,
                            ALU.add, ALU.max)
                        psh = psA.tile([64, 512], F32, tag="h")
                        nc.tensor.matmul(
                            out=psh[:], lhsT=whl_sb[:, 64 * t:64 * t + 64],
                            rhs=m1[:], start=True, stop=True)
                        m2 = m2h[st % 2]
                        nc.scalar.activation(
                            out=m2[0:64, :], in_=psh[:], func=AF.Relu,
                            bias=bh_sb[:, t:t + 1])
                        psm3 = psA.tile([128, 512], F32, tag="m3")
                        for j in range(ST):
                            nc.tensor.matmul(
                                out=psm3[:, 128 * j:128 * j + 128],
                                lhsT=m2[:, 128 * j:128 * j + 128],
                                rhs=wouth_sb[:, 128 * t:128 * t + 128],
                                start=True, stop=True)
                        m3 = epool.tile([128, 512], BF, tag="m3s")
                        nc.scalar.activation(out=m3[:], in_=psm3[:], func=AF.Relu)

                        for j in range(ST):
                            ch = ST * st + j
                            b = min(ch // cpb, c.NB - 1)
                            first = ch == b * cpb
                            last = (ch == n_chunks - 1 if b == c.NB - 1
                                    else ch == (b + 1) * cpb - 1)
                            oh = ohpool.tile([128, 128], BF, tag="oh")
                            nc.vector.tensor_scalar(
                                oh[:], iota128[:], drel_sb[:, ch:ch + 1], None,
                                ALU.is_equal)
                            if first:
                                psScur = psS.tile([128, 128], F32, tag="S")
                            nc.tensor.matmul(
                                out=psScur[:],
                                lhsT=m3[:, 128 * j:128 * j + 128],
                                rhs=oh[:], start=first, stop=last)
                            if last:
                                if t < S - 1:
                                    nc.vector.tensor_copy(
                                        out=sT[:, 128 * b:128 * b + 128],
                                        in_=psScur[:])
                                else:
                                    stb = bpool.tile([128, 128], BF, tag="stb")
                                    nc.vector.tensor_copy(out=stb[:], in_=psScur[:])
                                    psT = psA.tile([128, 128], BF, tag="h")
                                    nc.tensor.transpose(
                                        out=psT[:], in_=stb[:], identity=identity[:])
                                    sab = bpool.tile([128, 128], BF, tag="sab")
                                    nc.vector.tensor_copy(out=sab[:], in_=psT[:])
                                    moh = bpool.tile([128, c.MOLS], BF, tag="moh")
                                    nc.vector.tensor_scalar(
                                        moh[:], iota256[:, :c.MOLS],
                                        mols_sb[:, b:b + 1], None, ALU.is_equal)
                                    nc.tensor.matmul(
                                        out=psmol[:], lhsT=sab[:], rhs=moh[:],
                                        start=(b == 0), stop=(b == c.NB - 1))

                # ---------------- final MLP (replicated)
                molp = bpool.tile([128, c.MOLS], F32, tag="molp")
                nc.vector.tensor_copy(out=molp[:], in_=psmol[:])
                nc.sync.dma_start(out=molpart[:], in_=molp[:])
                if collectives:
                    nc.gpsimd.collective_compute(
                        "AllReduce", ALU.add,
                        replica_groups=[list(range(NCORES))],
                        ins=[molpart[:]], outs=[molfull[:]])
                else:
                    nc.sync.dma_start(out=molfull[:], in_=molpart[:])
                mol_bf = bpool.tile([128, c.MOLS], BF, tag="molbf")
                nc.gpsimd.dma_start(out=mol_bf[:], in_=molfull[:])
                psh1 = psA.tile([64, c.MOLS], F32, tag="h")
                nc.tensor.matmul(out=psh1[:], lhsT=fc1w_sb[:], rhs=mol_bf[:],
                                 start=True, stop=True)
                h1 = bpool.tile([64, c.MOLS], BF, tag="h1")
                nc.scalar.activation(out=h1[:], in_=psh1[:], func=AF.Relu,
                                     bias=fc1b_sb[:])
                psh2 = psA.tile([64, c.MOLS], F32, tag="h")
                nc.tensor.matmul(out=psh2[:], lhsT=fc2w_sb[:], rhs=h1[:],
                                 start=True, stop=True)
                h2 = bpool.tile([64, c.MOLS], BF, tag="h2")
                nc.scalar.activation(out=h2[:], in_=psh2[:], func=AF.Relu,
                                     bias=fc2b_sb[:])
                pso = psA.tile([32, c.MOLS], F32, tag="h")
                nc.tensor.matmul(out=pso[:], lhsT=outw_sb[:], rhs=h2[:],
                                 start=True, stop=True)
                osb = bpool.tile([32, c.MOLS], F32, tag="osb")
                nc.vector.tensor_scalar(osb[:], pso[:], outb_sb[:], None, ALU.add)
                nc.sync.dma_start(out=y[:], in_=osb[:])

    nc.compile()
    return nc


# ---------------------------------------------------------------- entry

_CACHE = {}


def _get_program(cfg, cpb, n_chunks):
    key = (cfg.N, cfg.E, cfg.MOLS, cpb, n_chunks)
    if key not in _CACHE:
        _CACHE[key] = build_program(cfg, cpb, n_chunks)
    return _CACHE[key]


def kernel(states, Win, b_in, Wh, b_h, Wout, b_out,
           fc1_w, fc1_b, fc2_w, fc2_b, out_w, out_b,
           src, dst, mol_ids, num_mols):
    cfg = Cfg(states.shape[0], src.shape[0], int(num_mols),
              steps=np.asarray(Win).shape[0])
    in_maps, cpb, n_chunks = preprocess(
        cfg, states, Win, b_in, Wh, b_h, Wout, b_out,
        fc1_w, fc1_b, fc2_w, fc2_b, out_w, out_b, src, dst, mol_ids)
    nc = _get_program(cfg, cpb, n_chunks)
    res = run_bass_kernel_spmd(nc, in_maps, core_ids=list(range(NCORES)))
    return np.ascontiguousarray(res.results[0]["y"].T)
